# revision 7
# baseline (speedup 1.0000x reference)
"""ChebGraphConv (K=3) Trainium2 kernel.

y = x@(W0-W2) - (A@x)@W1 + 2*A@((A@x)@W2) + bias

computed per (b,t) slice as:
  P0 = X@W02 ; P1 = X@W1 ; P2' = X@(2*W2)
  Q' = A@P2' ; M = P1 - Q' ; S = A@M ; y = P0 - S (+bias)

The spmms are fp8e5m2 DoubleRow matmuls. With `--enable-ldw-opt=false`
(hardcoded in this toolchain) every matmul reloads its stationary, so each
DR matmul costs ~LDWEIGHTS(256 cols) ~= 209 ns regardless of free size; the
kernel is weight-load bound and the only lever is the matmul COUNT. All 12
slices of a core are therefore pushed through each A-pass at FD=512 (PSUM
bank cap): the 1536 moving columns of work (P2' 768 | M 768) are split into
three 512-wide pass tiles

  R0 = P2'[slices 0-7]
  R1 = [P2' slices 8-11 | M slices 0-3]
  R2 = [M slices 4-11]

and three A-passes p0/p1/p2 run chain(k): sum_mt A^T[k,mt] @ Rp[mt] with one
FD=512 matmul per stationary -> 3*128 = 384 DR matmuls instead of the
4*128 = 512 a 6-slice-group structure needs. Each pass reads a tile written
only by earlier phases (proj -> R0,R1; p0 -> R1,R2; p1 -> R2), so coarse
tile-dependency tracking introduces no false PE serialization.

PSUM evacuations: M = P1 - psum (fp8 out), y = P0 - psum (f32), both DVE
tensor_tensor reading PSUM; proj copies (P2' fp8 strided into R-tiles, P1/P0
bf16) split between DVE and Act. Data parallel over B: core b handles x[b];
y is written [N, T, C] per core and transposed on host.
"""

import numpy as np
import ml_dtypes

import concourse.bacc as bacc
import concourse.mybir as mybir
import concourse.tile as tile
from concourse import bass_utils

BF16 = ml_dtypes.bfloat16
FP8E5 = ml_dtypes.float8_e5m2

B, T, N, C = 8, 12, 2048, 64
NB = N // 128          # 16 node blocks
NMT = NB // 2          # 8 DoubleRow contraction slabs (256 nodes each)
FD = 512               # PSUM-bank-capped matmul free size (8 slices x 64)

_NC_CACHE = {}


def _build_nc(repeat=None, with_bias=False):
    """repeat=None: single-shot kernel (graded path). repeat=R: wraps the
    whole body in a hardware For loop running it R times (benchmarking)."""
    key = ("nc", repeat, with_bias)
    if key in _NC_CACHE:
        return _NC_CACHE[key]
    f32 = mybir.dt.float32
    bf16 = mybir.dt.bfloat16
    fp8 = mybir.dt.float8e5

    nc = bacc.Bacc("TRN2", target_bir_lowering=False, debug=False,
                   enable_asserts=False, num_devices=8)

    at_d = nc.dram_tensor("at8", [NB, 128, NMT, 256], fp8,
                          kind="ExternalInput")
    xs_d = nc.dram_tensor("xs", [T // 2, 128, N], bf16, kind="ExternalInput")
    wa_d = nc.dram_tensor("wa", [128, 3 * C], bf16, kind="ExternalInput")
    bias_d = nc.dram_tensor("biasb", [128, C], f32, kind="ExternalInput")
    y_d = nc.dram_tensor("y", [N, T, C], f32, kind="ExternalOutput")

    with tile.TileContext(nc) as tc:
        with (
            tc.tile_pool(name="const", bufs=1) as constp,
            tc.tile_pool(name="atp", bufs=1) as atp,
            tc.tile_pool(name="xsp", bufs=1) as xsp,
            tc.tile_pool(name="rp", bufs=1) as rp,
            tc.tile_pool(name="pp10", bufs=2) as pp10,
            tc.tile_pool(name="ystage", bufs=4) as ystage,
            tc.tile_pool(name="pps", bufs=2, space="PSUM") as pps,
            tc.tile_pool(name="sps", bufs=4, space="PSUM") as sps,
        ):
            def emit_body():
                _emit(nc, constp, atp, xsp, rp, pp10, ystage, pps, sps,
                      at_d, xs_d, wa_d, bias_d, y_d, with_bias)

            if repeat is None:
                emit_body()
            else:
                with tc.For_i(0, repeat, 1):
                    emit_body()

    nc.compile()
    _NC_CACHE[key] = nc
    return nc


def _emit(nc, constp, atp, xsp, rp, pp10, ystage, pps, sps,
          at_d, xs_d, wa_d, bias_d, y_d, with_bias):
    f32 = mybir.dt.float32
    bf16 = mybir.dt.bfloat16
    fp8 = mybir.dt.float8e5

    wa_t = constp.tile([128, 3 * C], bf16, tag="wa")
    bias_t = constp.tile([128, C], f32, tag="bias")

    xs_t = [xsp.tile([128, N], bf16, tag=f"xs{p}", name=f"xs{p}")
            for p in range(T // 2)]
    at_t = [atp.tile([128, NMT, 256], fp8, tag=f"at{k}", name=f"at{k}")
            for k in range(NB)]
    # pass tiles: [128, slab, plane, 512]; R0=P2'[0-8), R1=[P2'[8-12)|M[0-4)],
    # R2=M[4-12). plane = node-block parity within the 256-node slab.
    r_t = [rp.tile([128, NMT, 2, FD], fp8, tag=f"r{p}", name=f"r{p}")
           for p in range(3)]
    # [128, k-block, {P1,P0}, slice*C] bf16 staging for the evac subtrahends
    p10_t = pp10.tile([128, NB, 2, T * C], bf16, tag="p10", name="p10")

    nc.sync.dma_start(xs_t[0][:], xs_d[0, :, :])
    nc.sync.dma_start(wa_t[:], wa_d[:, :])
    for p in range(1, T // 2):
        nc.sync.dma_start(xs_t[p][:], xs_d[p, :, :])
    for k in range(NB):
        nc.sync.dma_start(at_t[k][:], at_d[k, :, :, :])
    nc.sync.dma_start(bias_t[:], bias_d[:, :])

    def xstat(s, k):
        """Stationary [64, 128] for slice s, node block k."""
        return xs_t[s // 2][(s % 2) * C:(s % 2 + 1) * C, k * 128:(k + 1) * 128]

    def proj_step(s, q):
        """Projection for slice s, node blocks 4q..4q+3.
        pp columns per block j (at 256-col spacing): [P1|P2'|P0|pad]."""
        h = (s % 2) * C
        pp = pps.tile([128, 1024], f32, tag="pp", name="pp")
        for j in range(4):
            k = 4 * q + j
            nc.tensor.matmul(pp[:, j * 256:j * 256 + 192],
                             xstat(s, k),
                             wa_t[h:h + C, :], start=True, stop=True)
        pv = pp.rearrange("p (j2 i pl c) -> p j2 i pl c", j2=2, i=2, pl=4, c=C)
        # P2' -> pass tile (fp8): slices 0-7 -> R0, 8-11 -> R1 cols 0:256
        rt, s0 = (r_t[0], s) if s < 8 else (r_t[1], s - 8)
        rv = rt.rearrange("p m i (sl c) -> p m i sl c", sl=FD // C, c=C)
        # P1+P0 -> bf16 staging in one strided copy: view pp as
        # [j:4][pl2:2 (stride 128)][c:128] and take c<64 -> pl2=0 is P1,
        # pl2=1 is P0, matching p10_t's [k, {P1,P0}, cols] layout.
        p10v = pp.rearrange("p (j pl2 c) -> p j pl2 c", j=4, pl2=2, c=2 * C)
        eng_a, eng_b = ((nc.vector.tensor_copy, nc.scalar.copy)
                        if (s * 4 + q) % 2 == 0 else
                        (nc.scalar.copy, nc.vector.tensor_copy))
        eng_a(rv[:, 2 * q:2 * q + 2, :, s0, :], pv[:, :, :, 1, :])
        eng_b(p10_t[:, 4 * q:4 * q + 4, :, s * C:(s + 1) * C],
              p10v[:, :, :, 0:C])

    def interleave_emit(interleave, k, total_k):
        if not interleave:
            return
        nchunk = len(interleave)
        c0 = k * nchunk // total_k
        c1 = (k + 1) * nchunk // total_k
        for thunk in interleave[c0:c1]:
            thunk()

    def chain(p, k):
        """sp = A[kblk,:] @ Rp  (full 2048 contraction, FD=512)."""
        sp = sps.tile([128, FD], f32, tag="sp", name="sp")
        for mt in range(NMT):
            nc.tensor.matmul(sp[:], at_t[k][:, mt, :],
                             r_t[p][:, mt, :, :],
                             start=(mt == 0), stop=(mt == NMT - 1),
                             perf_mode=mybir.MatmulPerfMode.DoubleRowSwInterleave)
        return sp

    def m_dst(k, s0, ns):
        """R-tile destination for M slices [s0, s0+ns): slices 0-3 -> R1
        cols 256:512, slices 4-11 -> R2."""
        if s0 < 4:
            return r_t[1][:, k // 2, k % 2, 256 + s0 * C:256 + (s0 + ns) * C]
        return r_t[2][:, k // 2, k % 2, (s0 - 4) * C:(s0 - 4 + ns) * C]

    def emit_y(k, s0, ns, src):
        yt = ystage.tile([128, FD], f32, tag="y", name="yt")
        nc.vector.tensor_sub(yt[:, :ns * C], p10_t[:, k, 1, s0 * C:(s0 + ns) * C],
                             src)
        if with_bias:
            for i in range(ns):
                ysl = yt[:, i * C:(i + 1) * C]
                nc.vector.tensor_tensor(ysl, ysl, bias_t[:],
                                        op=mybir.AluOpType.add)
        nc.sync.dma_start(y_d[k * 128:(k + 1) * 128, s0:s0 + ns, :],
                          yt[:, :ns * C])

    # proj slices 0-7 (fill R0 + P1/P0); slices 8-11 interleave into pass 0
    for s in range(8):
        for q in range(NB // 4):
            proj_step(s, q)
    proj_rest = [(lambda ss=s, qq=q: proj_step(ss, qq))
                 for s in range(8, T) for q in range(NB // 4)]

    # pass 0: spmm2 slices 0-7 -> M[0-4) in R1, M[4-8) in R2
    for k in range(NB):
        interleave_emit(proj_rest, k, NB)
        sp = chain(0, k)
        nc.vector.tensor_tensor(m_dst(k, 0, 4), p10_t[:, k, 0, 0:4 * C],
                                sp[:, 0:4 * C], op=mybir.AluOpType.subtract)
        nc.vector.tensor_tensor(m_dst(k, 4, 4), p10_t[:, k, 0, 4 * C:8 * C],
                                sp[:, 4 * C:8 * C], op=mybir.AluOpType.subtract)

    # pass 1: spmm2 slices 8-11 -> M[8-12) in R2 ; spmm3 slices 0-3 -> y
    for k in range(NB):
        sp = chain(1, k)
        nc.vector.tensor_tensor(m_dst(k, 8, 4), p10_t[:, k, 0, 8 * C:12 * C],
                                sp[:, 0:4 * C], op=mybir.AluOpType.subtract)
        emit_y(k, 0, 4, sp[:, 4 * C:8 * C])

    # pass 2: spmm3 slices 4-11 -> y
    for k in range(NB):
        sp = chain(2, k)
        emit_y(k, 4, 8, sp[:])


def _prep_inputs(x, A_norm, weight, bias):
    """Host-side shard + layout prep. Returns per-core input maps."""
    x = np.asarray(x, dtype=np.float32)
    A_norm = np.asarray(A_norm, dtype=np.float32)
    weight = np.asarray(weight, dtype=np.float32)
    bias = np.asarray(bias, dtype=np.float32)

    # per-k-block DoubleRowSwInterleave A^T pack: for each (kb, mt) the
    # 256-wide stationary holds [A127,B127,A126,B126,...,A0,B0] per
    # partition kp, where A/B = planes i=0/1 and column n' runs reversed:
    # at8[kb, kp, mt, 2*(127-n')+i] = A[kb*128+n', mt*256 + i*128 + kp]
    A2 = A_norm.reshape(NB, 128, NMT, 2, 128)        # [kb, n', mt, i, kp]
    at8 = A2.transpose(0, 4, 2, 1, 3)[:, :, :, ::-1, :]  # [kb, kp, mt, n'r, i]
    at8_host = np.ascontiguousarray(at8.reshape(NB, 128, NMT, 256)).astype(FP8E5)

    W0, W1, W2 = weight[0], weight[1], weight[2]
    wa_host = np.zeros((128, 3 * C), dtype=BF16)
    for h in (0, C):
        wa_host[h:h + C, 0:C] = W1.astype(BF16)
        wa_host[h:h + C, C:2 * C] = (2.0 * W2).astype(BF16)
        wa_host[h:h + C, 2 * C:3 * C] = (W0 - W2).astype(BF16)

    bias_host = np.ascontiguousarray(np.broadcast_to(bias, (128, C)),
                                     dtype=np.float32)

    in_maps = []
    for b in range(B):
        xt = x[b].transpose(0, 2, 1)                 # [T, C, N]
        xt = xt.reshape(T // 2, 128, N)              # pair slices on partitions
        in_maps.append({
            "at8": at8_host,
            "xs": np.ascontiguousarray(xt).astype(BF16),
            "wa": wa_host,
            "biasb": bias_host,
        })
    return in_maps


def kernel(x, A_norm, weight, bias):
    with_bias = bool(np.any(np.asarray(bias)))
    nc = _build_nc(with_bias=with_bias)
    in_maps = _prep_inputs(x, A_norm, weight, bias)
    last_err = None
    for attempt in range(3):
        try:
            res = bass_utils.run_bass_kernel_spmd(nc, in_maps,
                                                  core_ids=list(range(8)))
            break
        except Exception as e:  # transient NRT_EXEC_UNIT_UNRECOVERABLE etc.
            last_err = e
            import time
            time.sleep(2.0 * (attempt + 1))
    else:
        raise last_err
    # per-core y is [N, T, C]; full output is [B, T, N, C]
    out = np.stack([res.results[b]["y"].transpose(1, 0, 2) for b in range(B)],
                   axis=0)
    return np.ascontiguousarray(out, dtype=np.float32)


# revision 11
# speedup vs baseline: 1.3641x; 1.3641x over previous
"""ChebGraphConv (K=3) Trainium2 kernel.

y = x@(W0-W2) - (A@x)@W1 + 2*A@((A@x)@W2) + bias

computed per (b,t) slice as:
  P0 = X@W02 ; P1 = X@W1 ; P2' = X@(2*W2)   (one 192-wide bf16 matmul per
                                             node block, X bf16, no hi/lo)
  Q' = A@P2' ; M = P1 - Q' ; S = A@M ; y = P0 - S (+bias)

A is stored in fp8e5m2 at natural scale (entries ~U(0,1/2048) sit in e5m2's
normal range), so the two spmms run as fp8 DoubleRow matmuls with NO descale
op: each spmm k-block needs exactly one DVE subtract to evacuate PSUM.
P1/P2'/M are e5m2 (they only reach y through the small A@(...) terms,
|A@v| ~ 0.02*|v|); P0 is bf16 (dominant term). Measured numpy rel err ~5e-3
vs the 2e-2 gate.

Projections write 4 node blocks per PSUM tile (2 banks, 256-col spacing) so
evacuation is 2 ops per 4 blocks (one fp8 p12 copy, one bf16 P0 copy),
alternated between DVE and Act to balance engine load.

Data parallel over B: core b handles x[b] (T=12 slices), 2 groups of 6
slices; group 1's projections interleave into group 0's spmm phases. y is
written [N, T, C] per core (contiguous DMA) and transposed on host.
"""

import numpy as np
import ml_dtypes

import concourse.bacc as bacc
import concourse.mybir as mybir
import concourse.tile as tile
from concourse import bass_utils

BF16 = ml_dtypes.bfloat16
FP8E5 = ml_dtypes.float8_e5m2

B, T, N, C = 8, 12, 2048, 64
NB = N // 128          # 16 node blocks
NMT = NB // 2          # 8 DoubleRow contraction tiles (256 nodes each)
G = 6                  # slices per group
GW = G * C             # 384

_NC_CACHE = {}


def _build_nc(repeat=None, with_bias=False):
    """repeat=None: single-shot kernel (graded path). repeat=R: wraps the
    whole body in a hardware For loop running it R times (benchmarking)."""
    key = ("nc", repeat, with_bias)
    if key in _NC_CACHE:
        return _NC_CACHE[key]
    f32 = mybir.dt.float32
    bf16 = mybir.dt.bfloat16
    fp8 = mybir.dt.float8e5

    nc = bacc.Bacc("TRN2", target_bir_lowering=False, debug=False,
                   enable_asserts=False, num_devices=8)

    at_d = nc.dram_tensor("at8", [NB // 4, 128, 4, NMT, 256], fp8,
                          kind="ExternalInput")
    xsa_d = nc.dram_tensor("xsa", [128, 2, N], bf16, kind="ExternalInput")
    xsb_d = nc.dram_tensor("xsb", [128, 4, N], bf16, kind="ExternalInput")
    wa_d = nc.dram_tensor("wa", [128, 3 * C], bf16, kind="ExternalInput")
    bias_d = nc.dram_tensor("biasb", [128, C], f32, kind="ExternalInput")
    y_d = nc.dram_tensor("y", [N, T, C], f32, kind="ExternalOutput")

    with tile.TileContext(nc) as tc:
        with (
            tc.tile_pool(name="const", bufs=1) as constp,
            tc.tile_pool(name="atp", bufs=2) as atp,
            tc.tile_pool(name="xsp", bufs=2) as xsp,
            tc.tile_pool(name="p12p", bufs=2) as p12p,
            tc.tile_pool(name="p0p", bufs=2) as p0p,
            tc.tile_pool(name="mp", bufs=2) as mp,
            tc.tile_pool(name="ystage", bufs=3) as ystage,
            tc.tile_pool(name="pps", bufs=2, space="PSUM") as pps,
            tc.tile_pool(name="sps", bufs=4, space="PSUM") as sps,
        ):
            def emit_body():
                _emit(nc, constp, atp, xsp, p12p, p0p, mp, ystage, pps, sps,
                      at_d, xsa_d, xsb_d, wa_d, bias_d, y_d, with_bias)

            if repeat is None:
                emit_body()
            else:
                with tc.For_i(0, repeat, 1):
                    emit_body()

    nc.compile()
    _NC_CACHE[key] = nc
    return nc


def _emit(nc, constp, atp, xsp, p12p, p0p, mp, ystage, pps, sps,
          at_d, xsa_d, xsb_d, wa_d, bias_d, y_d, with_bias):
    f32 = mybir.dt.float32
    bf16 = mybir.dt.bfloat16
    fp8 = mybir.dt.float8e5

    wa_t = constp.tile([128, 3 * C], bf16, tag="wa")
    bias_t = constp.tile([128, C], f32, tag="bias")

    # one xs tile per slice-pair: [128, N] with partitions = 2 slices x 64 c,
    # and one A^T tile per output node block k (holding all 8 DoubleRow
    # stationaries for that block). DMAs all serialize on the shared DMA
    # engines, so the issue ORDER is chosen to match consumption order:
    # proj pair p needs xs_t[p], spmm chain k needs only at_t[k].
    xs_t2 = [xsp.tile([128, 2, N], bf16, tag="xsa", name="xsa"),
             xsp.tile([128, 4, N], bf16, tag="xsb", name="xsb")]
    at_t = [atp.tile([128, 4, NMT, 256], fp8, tag=f"at{g}", name=f"at{g}")
            for g in range(NB // 4)]
    nc.sync.dma_start(xs_t2[0][:], xsa_d[:, :, :])
    nc.sync.dma_start(wa_t[:], wa_d[:, :])
    nc.sync.dma_start(xs_t2[1][:], xsb_d[:, :, :])
    for g in range(NB // 4):
        nc.sync.dma_start(at_t[g][:], at_d[g, :, :, :, :])
    nc.sync.dma_start(bias_t[:], bias_d[:, :])

    def xstat(g, idx, k):
        """Stationary [64, 128] for slice idx of group g, node block k."""
        t = g * G + idx
        p = t // 2
        xt, po = (xs_t2[0], p) if p < 2 else (xs_t2[1], p - 2)
        return xt[(t % 2) * C:(t % 2 + 1) * C, po, k * 128:(k + 1) * 128]

    def proj_step(g, p12, p0, idx, q):
        """Projection for slice idx of group g, node blocks 4q..4q+3.
        pp columns per block j (at 256-col spacing): [P1|P2'|P0|pad]."""
        cs = slice(idx * C, (idx + 1) * C)
        pp = pps.tile([128, 1024], f32, tag="pp", name="pp")
        for j in range(4):
            k = 4 * q + j
            h = ((g * G + idx) % 2) * C
            nc.tensor.matmul(pp[:, j * 256:j * 256 + 192],
                             xstat(g, idx, k),
                             wa_t[h:h + C, :], start=True, stop=True)
        pv = pp.rearrange("p (j pl c) -> p j pl c", j=4, pl=4, c=C)
        s = idx * 4 + q
        # p12 copy: [128, 4(k), 2(pl), 64] fp8; P0 copy: [128, 4(k), 64] bf16
        p12_dst = p12[q][:, :, :, cs]
        p0_dst = p0[q][:, :, cs]
        if g == 1:
            # interleaved steps: p12 on Act, P0 on DVE (DVE also carries the
            # spmm subtracts; this split evens both at ~1 op per k-window)
            nc.scalar.copy(p12_dst, pv[:, :, 0:2, :])
            nc.vector.tensor_copy(p0_dst, pv[:, :, 2, :])
        elif s % 2 == 0:
            nc.vector.tensor_copy(p12_dst, pv[:, :, 0:2, :])
            nc.scalar.copy(p0_dst, pv[:, :, 2, :])
        else:
            nc.scalar.copy(p12_dst, pv[:, :, 0:2, :])
            nc.vector.tensor_copy(p0_dst, pv[:, :, 2, :])

    def dr_chain(sp, k, moving):
        """One fp8 DoubleRow accumulation chain: sp += A^T[kblk] @ moving.
        moving: [128, 2, GW] fp8 slices of a [128, NB, GW]-shaped view."""
        for mt in range(NMT):
            nc.tensor.matmul(sp[:], at_t[k // 4][:, k % 4, mt, :],
                             moving(mt),
                             start=(mt == 0), stop=(mt == NMT - 1),
                             perf_mode=mybir.MatmulPerfMode.DoubleRowSwInterleave)

    def interleave_emit(interleave, k, total_k):
        if not interleave:
            return
        nchunk = len(interleave)
        c0 = k * nchunk // total_k
        c1 = (k + 1) * nchunk // total_k
        for thunk in interleave[c0:c1]:
            thunk()

    def spmm2(p12, m, interleave=None, ilv_k0=0):
        """Q' = A@P2' ; M = P1 - Q' (one DVE subtract per k-block)."""
        for k in range(NB):
            if k >= ilv_k0:
                interleave_emit(interleave, k - ilv_k0, NB - ilv_k0)
            sp = sps.tile([128, GW], f32, tag="sp", name="sp")
            dr_chain(sp, k, lambda mt: p12[mt // 2][:, 2 * (mt % 2):
                                                    2 * (mt % 2) + 2, 1, :])
            nc.vector.tensor_tensor(m[:, k, :], p12[k // 4][:, k % 4, 0, :],
                                    sp[:], op=mybir.AluOpType.subtract)

    def spmm3(m, p0, s0, interleave=None):
        """S = A@M ; y = P0 - S (+bias)."""
        for k in range(NB):
            interleave_emit(interleave, k, NB)
            sp = sps.tile([128, GW], f32, tag="sp", name="sp")
            dr_chain(sp, k, lambda mt: m[:, 2 * mt:2 * mt + 2, :])
            yt = ystage.tile([128, GW], f32, tag="y", name="yt")
            nc.vector.tensor_sub(yt[:], p0[k // 4][:, k % 4, :], sp[:])
            if with_bias:
                for idx in range(G):
                    ysl = yt[:, idx * C:(idx + 1) * C]
                    nc.vector.tensor_tensor(ysl, ysl, bias_t[:],
                                            op=mybir.AluOpType.add)
            (nc.sync if k % 2 == 0 else nc.gpsimd).dma_start(
                y_d[k * 128:(k + 1) * 128, s0:s0 + G, :], yt[:])

    # per-q tiles (4 node blocks each) so spmm chains can start as soon as
    # the first q's projections land; p12 layout [128, 4(k), 2(pl), GW]
    def group_tiles(g):
        p12 = [p12p.tile([128, 4, 2, GW], fp8, tag=f"p12q{q}",
                         name=f"p12_{g}q{q}") for q in range(4)]
        p0 = [p0p.tile([128, 4, GW], bf16, tag=f"p0q{q}",
                       name=f"p0_{g}q{q}") for q in range(4)]
        m = mp.tile([128, NB, GW], fp8, tag="m", name=f"m_{g}")
        return p12, p0, m

    p12_0, p0_0, m_0 = group_tiles(0)
    for q in range(NB // 4):
        for idx in range(G):
            proj_step(0, p12_0, p0_0, idx, q)

    # group 1 proj rides inside group 0's spmm phases (its PSUM evacuation
    # hides under the spmm chains); needs p12/p0 bufs=2
    p12_1, p0_1, m_1 = group_tiles(1)
    proj1 = [(lambda i=idx, qq=q: proj_step(1, p12_1, p0_1, i, qq))
             for q in range(NB // 4) for idx in range(G)]
    spmm2(p12_0, m_0, interleave=proj1[:8], ilv_k0=8)
    spmm3(m_0, p0_0, 0, interleave=proj1[8:])

    spmm2(p12_1, m_1)
    spmm3(m_1, p0_1, G)


def _prep_inputs(x, A_norm, weight, bias):
    """Host-side shard + layout prep. Returns per-core input maps."""
    x = np.asarray(x, dtype=np.float32)
    A_norm = np.asarray(A_norm, dtype=np.float32)
    weight = np.asarray(weight, dtype=np.float32)
    bias = np.asarray(bias, dtype=np.float32)

    # per-k-block DoubleRowSwInterleave A^T pack: for each (kb, mt) the
    # 256-wide stationary holds [A127,B127,A126,B126,...,A0,B0] per
    # partition kp, where A/B = planes i=0/1 and column n' runs reversed:
    # at8[kb, kp, mt, 2*(127-n')+i] = A[kb*128+n', mt*256 + i*128 + kp]
    A2 = A_norm.reshape(NB, 128, NMT, 2, 128)        # [kb, n', mt, i, kp]
    at8 = A2.transpose(0, 4, 2, 1, 3)[:, :, :, ::-1, :]  # [kb, kp, mt, n'r, i]
    at8_host = np.ascontiguousarray(
        at8.reshape(NB // 4, 4, 128, NMT, 256).transpose(0, 2, 1, 3, 4)
    ).astype(FP8E5)

    W0, W1, W2 = weight[0], weight[1], weight[2]
    wa_host = np.zeros((128, 3 * C), dtype=BF16)
    for h in (0, C):
        wa_host[h:h + C, 0:C] = W1.astype(BF16)
        wa_host[h:h + C, C:2 * C] = (2.0 * W2).astype(BF16)
        wa_host[h:h + C, 2 * C:3 * C] = (W0 - W2).astype(BF16)

    bias_host = np.ascontiguousarray(np.broadcast_to(bias, (128, C)),
                                     dtype=np.float32)

    in_maps = []
    for b in range(B):
        xt = x[b].transpose(0, 2, 1)                 # [T, C, N]
        xt = xt.reshape(T // 2, 128, N)              # pair slices on partitions
        in_maps.append({
            "at8": at8_host,
            "xsa": np.ascontiguousarray(xt[0:2].transpose(1, 0, 2)).astype(BF16),
            "xsb": np.ascontiguousarray(xt[2:6].transpose(1, 0, 2)).astype(BF16),
            "wa": wa_host,
            "biasb": bias_host,
        })
    return in_maps


def kernel(x, A_norm, weight, bias):
    with_bias = bool(np.any(np.asarray(bias)))
    nc = _build_nc(with_bias=with_bias)
    in_maps = _prep_inputs(x, A_norm, weight, bias)
    last_err = None
    for attempt in range(3):
        try:
            res = bass_utils.run_bass_kernel_spmd(nc, in_maps,
                                                  core_ids=list(range(8)))
            break
        except Exception as e:  # transient NRT_EXEC_UNIT_UNRECOVERABLE etc.
            last_err = e
            import time
            time.sleep(2.0 * (attempt + 1))
    else:
        raise last_err
    # per-core y is [N, T, C]; full output is [B, T, N, C]
    out = np.stack([res.results[b]["y"].transpose(1, 0, 2) for b in range(B)],
                   axis=0)
    return np.ascontiguousarray(out, dtype=np.float32)



# revision 12
# speedup vs baseline: 1.4384x; 1.0544x over previous
"""ChebGraphConv (K=3) Trainium2 kernel.

y = x@(W0-W2) - (A@x)@W1 + 2*A@((A@x)@W2) + bias

computed per (b,t) slice as:
  P0 = X@W02 ; P1 = X@W1 ; P2' = X@(2*W2)   (one 192-wide bf16 matmul per
                                             node block, X bf16, no hi/lo)
  Q' = A@P2' ; M = P1 - Q' ; S = A@M ; y = P0 - S (+bias)

A is stored in fp8e5m2 at natural scale (entries ~U(0,1/2048) sit in e5m2's
normal range), so the two spmms run as fp8 DoubleRow matmuls with NO descale
op: each spmm k-block needs exactly one DVE subtract to evacuate PSUM.
P1/P2'/M are e5m2 (they only reach y through the small A@(...) terms,
|A@v| ~ 0.02*|v|); P0 is bf16 (dominant term). Measured numpy rel err ~5e-3
vs the 2e-2 gate.

Projections write 4 node blocks per PSUM tile (2 banks, 256-col spacing) so
evacuation is 2 ops per 4 blocks (one fp8 p12 copy, one bf16 P0 copy),
alternated between DVE and Act to balance engine load.

Data parallel over B: core b handles x[b] (T=12 slices), 2 groups of 6
slices; group 1's projections interleave into group 0's spmm phases. y is
written [N, T, C] per core (contiguous DMA) and transposed on host.
"""

import numpy as np
import ml_dtypes

import concourse.bacc as bacc
import concourse.mybir as mybir
import concourse.tile as tile
from concourse import bass_utils

BF16 = ml_dtypes.bfloat16
FP8E5 = ml_dtypes.float8_e5m2

B, T, N, C = 8, 12, 2048, 64
NB = N // 128          # 16 node blocks
NMT = NB // 2          # 8 DoubleRow contraction tiles (256 nodes each)
G = 6                  # slices per group
GW = G * C             # 384

_NC_CACHE = {}


def _build_nc(repeat=None, with_bias=False):
    """repeat=None: single-shot kernel (graded path). repeat=R: wraps the
    whole body in a hardware For loop running it R times (benchmarking)."""
    key = ("nc", repeat, with_bias)
    if key in _NC_CACHE:
        return _NC_CACHE[key]
    f32 = mybir.dt.float32
    bf16 = mybir.dt.bfloat16
    fp8 = mybir.dt.float8e5

    nc = bacc.Bacc("TRN2", target_bir_lowering=False, debug=False,
                   enable_asserts=False, num_devices=8)

    at_d = nc.dram_tensor("at8", [NB, 128, NMT, 256], fp8,
                          kind="ExternalInput")
    xs_d = nc.dram_tensor("xs", [T // 2, 128, N], bf16, kind="ExternalInput")
    wa_d = nc.dram_tensor("wa", [128, 3 * C], bf16, kind="ExternalInput")
    bias_d = nc.dram_tensor("biasb", [128, C], f32, kind="ExternalInput")
    y_d = nc.dram_tensor("y", [N, T, C], f32, kind="ExternalOutput")

    with tile.TileContext(nc) as tc:
        with (
            tc.tile_pool(name="const", bufs=1) as constp,
            tc.tile_pool(name="atp", bufs=2) as atp,
            tc.tile_pool(name="xsp", bufs=2) as xsp,
            tc.tile_pool(name="p12p", bufs=2) as p12p,
            tc.tile_pool(name="p0p", bufs=2) as p0p,
            tc.tile_pool(name="mp", bufs=2) as mp,
            tc.tile_pool(name="ystage", bufs=3) as ystage,
            tc.tile_pool(name="pps", bufs=2, space="PSUM") as pps,
            tc.tile_pool(name="sps", bufs=4, space="PSUM") as sps,
        ):
            def emit_body():
                _emit(nc, constp, atp, xsp, p12p, p0p, mp, ystage, pps, sps,
                      at_d, xs_d, wa_d, bias_d, y_d, with_bias)

            if repeat is None:
                emit_body()
            else:
                with tc.For_i(0, repeat, 1):
                    emit_body()

    nc.compile()
    _NC_CACHE[key] = nc
    return nc


def _emit(nc, constp, atp, xsp, p12p, p0p, mp, ystage, pps, sps,
          at_d, xs_d, wa_d, bias_d, y_d, with_bias):
    f32 = mybir.dt.float32
    bf16 = mybir.dt.bfloat16
    fp8 = mybir.dt.float8e5

    wa_t = constp.tile([128, 3 * C], bf16, tag="wa")
    bias_t = constp.tile([128, C], f32, tag="bias")

    # one xs tile per slice-pair: [128, N] with partitions = 2 slices x 64 c,
    # and one A^T tile per output node block k (holding all 8 DoubleRow
    # stationaries for that block). DMAs all serialize on the shared DMA
    # engines, so the issue ORDER is chosen to match consumption order:
    # proj pair p needs xs_t[p], spmm chain k needs only at_t[k].
    xs_t = [xsp.tile([128, N], bf16, tag=f"xs{p}", name=f"xs{p}")
            for p in range(T // 2)]
    at_t = [atp.tile([128, NMT, 256], fp8, tag=f"at{k}", name=f"at{k}")
            for k in range(NB)]
    nc.sync.dma_start(xs_t[0][:], xs_d[0, :, :])
    nc.sync.dma_start(wa_t[:], wa_d[:, :])
    nc.sync.dma_start(xs_t[1][:], xs_d[1, :, :])
    nc.sync.dma_start(xs_t[2][:], xs_d[2, :, :])
    nc.sync.dma_start(bias_t[:], bias_d[:, :])
    for k in range(NB):
        nc.sync.dma_start(at_t[k][:], at_d[k, :, :, :])
    for p in range(3, T // 2):
        nc.sync.dma_start(xs_t[p][:], xs_d[p, :, :])

    def xstat(g, idx, k):
        """Stationary [64, 128] for slice idx of group g, node block k."""
        t = g * G + idx
        return xs_t[t // 2][(t % 2) * C:(t % 2 + 1) * C, k * 128:(k + 1) * 128]

    def proj_step(g, p12, p0, idx, q):
        """Projection for slice idx of group g, node blocks 4q..4q+3.
        pp columns per block j (at 256-col spacing): [P1|P2'|P0|pad]."""
        cs = slice(idx * C, (idx + 1) * C)
        pp = pps.tile([128, 1024], f32, tag="pp", name="pp")
        for j in range(4):
            k = 4 * q + j
            h = ((g * G + idx) % 2) * C
            nc.tensor.matmul(pp[:, j * 256:j * 256 + 192],
                             xstat(g, idx, k),
                             wa_t[h:h + C, :], start=True, stop=True)
        pv = pp.rearrange("p (j pl c) -> p j pl c", j=4, pl=4, c=C)
        s = idx * 4 + q
        # p12 copy: [128, 4(k), 2(pl), 64] fp8; P0 copy: [128, 4(k), 64] bf16
        p12_dst = p12[q][:, :, :, cs]
        p0_dst = p0[q][:, :, cs]
        if g == 1:
            # interleaved steps: p12 on Act, P0 on DVE (DVE also carries the
            # spmm subtracts; this split evens both at ~1 op per k-window)
            nc.scalar.copy(p12_dst, pv[:, :, 0:2, :])
            nc.vector.tensor_copy(p0_dst, pv[:, :, 2, :])
        elif s % 2 == 0:
            nc.vector.tensor_copy(p12_dst, pv[:, :, 0:2, :])
            nc.scalar.copy(p0_dst, pv[:, :, 2, :])
        else:
            nc.scalar.copy(p12_dst, pv[:, :, 0:2, :])
            nc.vector.tensor_copy(p0_dst, pv[:, :, 2, :])

    def dr_chain(sp, k, moving):
        """One fp8 DoubleRow accumulation chain: sp += A^T[kblk] @ moving.
        moving: [128, 2, GW] fp8 slices of a [128, NB, GW]-shaped view."""
        for mt in range(NMT):
            nc.tensor.matmul(sp[:], at_t[k][:, mt, :],
                             moving(mt),
                             start=(mt == 0), stop=(mt == NMT - 1),
                             perf_mode=mybir.MatmulPerfMode.DoubleRowSwInterleave)

    def interleave_emit(interleave, k, total_k):
        if not interleave:
            return
        nchunk = len(interleave)
        c0 = k * nchunk // total_k
        c1 = (k + 1) * nchunk // total_k
        for thunk in interleave[c0:c1]:
            thunk()

    def spmm2(p12, m, interleave=None, ilv_k0=0):
        """Q' = A@P2' ; M = P1 - Q' (one DVE subtract per k-block)."""
        for k in range(NB):
            if k >= ilv_k0:
                interleave_emit(interleave, k - ilv_k0, NB - ilv_k0)
            sp = sps.tile([128, GW], f32, tag="sp", name="sp")
            dr_chain(sp, k, lambda mt: p12[mt // 2][:, 2 * (mt % 2):
                                                    2 * (mt % 2) + 2, 1, :])
            nc.vector.tensor_tensor(m[:, k, :], p12[k // 4][:, k % 4, 0, :],
                                    sp[:], op=mybir.AluOpType.subtract)

    def spmm3(m, p0, s0, interleave=None):
        """S = A@M ; y = P0 - S (+bias)."""
        for k in range(NB):
            interleave_emit(interleave, k, NB)
            sp = sps.tile([128, GW], f32, tag="sp", name="sp")
            dr_chain(sp, k, lambda mt: m[:, 2 * mt:2 * mt + 2, :])
            yt = ystage.tile([128, GW], f32, tag="y", name="yt")
            nc.vector.tensor_sub(yt[:], p0[k // 4][:, k % 4, :], sp[:])
            if with_bias:
                for idx in range(G):
                    ysl = yt[:, idx * C:(idx + 1) * C]
                    nc.vector.tensor_tensor(ysl, ysl, bias_t[:],
                                            op=mybir.AluOpType.add)
            nc.sync.dma_start(y_d[k * 128:(k + 1) * 128, s0:s0 + G, :], yt[:])

    # per-q tiles (4 node blocks each) so spmm chains can start as soon as
    # the first q's projections land; p12 layout [128, 4(k), 2(pl), GW]
    def group_tiles(g):
        p12 = [p12p.tile([128, 4, 2, GW], fp8, tag=f"p12q{q}",
                         name=f"p12_{g}q{q}") for q in range(4)]
        p0 = [p0p.tile([128, 4, GW], bf16, tag=f"p0q{q}",
                       name=f"p0_{g}q{q}") for q in range(4)]
        m = mp.tile([128, NB, GW], fp8, tag="m", name=f"m_{g}")
        return p12, p0, m

    p12_0, p0_0, m_0 = group_tiles(0)
    for q in range(NB // 4):
        for idx in range(G):
            proj_step(0, p12_0, p0_0, idx, q)

    # group 1 proj rides inside group 0's spmm phases (its PSUM evacuation
    # hides under the spmm chains); needs p12/p0 bufs=2
    p12_1, p0_1, m_1 = group_tiles(1)
    proj1 = [(lambda i=idx, qq=q: proj_step(1, p12_1, p0_1, i, qq))
             for q in range(NB // 4) for idx in range(G)]
    spmm2(p12_0, m_0, interleave=proj1[:8], ilv_k0=8)
    spmm3(m_0, p0_0, 0, interleave=proj1[8:])

    spmm2(p12_1, m_1)
    spmm3(m_1, p0_1, G)


def _prep_inputs(x, A_norm, weight, bias):
    """Host-side shard + layout prep. Returns per-core input maps."""
    x = np.asarray(x, dtype=np.float32)
    A_norm = np.asarray(A_norm, dtype=np.float32)
    weight = np.asarray(weight, dtype=np.float32)
    bias = np.asarray(bias, dtype=np.float32)

    # per-k-block DoubleRowSwInterleave A^T pack: for each (kb, mt) the
    # 256-wide stationary holds [A127,B127,A126,B126,...,A0,B0] per
    # partition kp, where A/B = planes i=0/1 and column n' runs reversed:
    # at8[kb, kp, mt, 2*(127-n')+i] = A[kb*128+n', mt*256 + i*128 + kp]
    A2 = A_norm.reshape(NB, 128, NMT, 2, 128)        # [kb, n', mt, i, kp]
    at8 = A2.transpose(0, 4, 2, 1, 3)[:, :, :, ::-1, :]  # [kb, kp, mt, n'r, i]
    at8_host = np.ascontiguousarray(at8.reshape(NB, 128, NMT, 256)).astype(FP8E5)

    W0, W1, W2 = weight[0], weight[1], weight[2]
    wa_host = np.zeros((128, 3 * C), dtype=BF16)
    for h in (0, C):
        wa_host[h:h + C, 0:C] = W1.astype(BF16)
        wa_host[h:h + C, C:2 * C] = (2.0 * W2).astype(BF16)
        wa_host[h:h + C, 2 * C:3 * C] = (W0 - W2).astype(BF16)

    bias_host = np.ascontiguousarray(np.broadcast_to(bias, (128, C)),
                                     dtype=np.float32)

    in_maps = []
    for b in range(B):
        xt = x[b].transpose(0, 2, 1)                 # [T, C, N]
        xt = xt.reshape(T // 2, 128, N)              # pair slices on partitions
        in_maps.append({
            "at8": at8_host,
            "xs": np.ascontiguousarray(xt).astype(BF16),
            "wa": wa_host,
            "biasb": bias_host,
        })
    return in_maps


def kernel(x, A_norm, weight, bias):
    with_bias = bool(np.any(np.asarray(bias)))
    nc = _build_nc(with_bias=with_bias)
    in_maps = _prep_inputs(x, A_norm, weight, bias)
    last_err = None
    for attempt in range(3):
        try:
            res = bass_utils.run_bass_kernel_spmd(nc, in_maps,
                                                  core_ids=list(range(8)))
            break
        except Exception as e:  # transient NRT_EXEC_UNIT_UNRECOVERABLE etc.
            last_err = e
            import time
            time.sleep(2.0 * (attempt + 1))
    else:
        raise last_err
    # per-core y is [N, T, C]; full output is [B, T, N, C]
    out = np.stack([res.results[b]["y"].transpose(1, 0, 2) for b in range(B)],
                   axis=0)
    return np.ascontiguousarray(out, dtype=np.float32)

